# revision 56
# baseline (speedup 1.0000x reference)
"""Causal self-attention (B=2, T=2048, C=1024, NH=16) on 8 trn2 NeuronCores.

Sharding: core c handles batch b = c//4 and head group g = c%4 (4 heads,
256 features). Each core computes q/k/v for its heads, causal attention in
S^T layout (keys on partitions, queries on the free dim), and a partial
output projection  y_heads @ w_proj[head_rows, :].  The host sums the four
partial projections per batch and adds b_proj.

v5 (309us -> ~158us; from the v3 trace: 128us of PE idle in ~24 gaps +
HAM half-clock for ~40% of the span):
- x is transposed on the HOST and DMA'd contiguously in query-chunk
  column order (the v3 XBAR transpose DMAs took ~29us and stalled the
  whole front of the kernel); weights are host-permuted so every input
  DMA is one contiguous block, split across the sync/scalar hwdge queues.
- The softmax denominator is REPLICATED across 64 PSUM partitions for
  free by widening the PV stationary to [V | ones*64] / [ones*64 | V]
  (matmul cost depends only on the moving free dim, not M).  The v3
  normalization chain (row-pack copies -> scalar Ln -> scalar Exp with
  two 1.3us activation-table reloads -> 2 gpsimd broadcast DMAs -> muls,
  ~8-12us serial per group) collapses to: plain partition-shifted DVE
  copy, reciprocal_approx_fast at base partition 0 (the custom op
  returns garbage at any other base partition), aligned muls.
- HAM (PE clock gate, 1.2 vs 2.4 GHz): a DMA-independent warm-up burst
  opens the activity window during NEFF init, keep-alive matmuls bridge
  the DMA-gated first QKV chains, and proj tiles are withheld and pumped
  (paced, with a reserve) through the exp-rate-limited last query chunk
  so the PE duty cycle never drops long enough to re-throttle mid-body.
- Causal masks split across vector+gpsimd; head B's scores are packed at
  [QCW, 2QCW-lo) so the per-unit exp range is contiguous with no junk.
Steady state measured: back-to-back matmuls at N/2.4GHz + ~3ns, HAM at
8/8 for the whole body, ~10us total PE idle.  The remaining time is the
NEFF init barrier (~7us), the ~17us DMA-bound input chase (6.2MB at the
~358GB/s per-core HBM limit), and the serial normalize->proj->DMA tail.
"""

import os
import sys

import numpy as np

for _p in ("/opt/trn_rl_repo", "/root/.axon_site/_ro/trn_rl_repo"):
    if _p not in sys.path and os.path.isdir(_p):
        sys.path.append(_p)

import concourse.bass as bass  # noqa: E402
import concourse.tile as tile  # noqa: E402
from concourse import bacc, mybir  # noqa: E402
from concourse.bass_utils import run_bass_kernel_spmd  # noqa: E402

P = 128
B, T, C = 2, 2048, 1024
NH, HD = 16, 64
HPC = 4  # heads per core
FPC = HPC * HD  # features per core (256)
QCW = 512  # query-chunk width
F32 = mybir.dt.float32
BF16 = mybir.dt.bfloat16
ADD = mybir.AluOpType.add
MULT = mybir.AluOpType.mult
EXP = mybir.ActivationFunctionType.Exp


def build_nc(t_len: int = T):
    """Build the per-core Bass program (same program on all 8 cores)."""
    nt = t_len // P  # token tiles (16)
    ncb = C // P  # contraction blocks (8)
    nqc = t_len // QCW  # query chunks (4)
    tpq = QCW // P  # token tiles per query chunk (4)

    nc = bacc.Bacc("TRN2", target_bir_lowering=False, debug=False)

    xT_d = nc.dram_tensor("xT", [C, t_len], BF16, kind="ExternalInput")
    wq_d = nc.dram_tensor("wq", [P, ncb, FPC], BF16, kind="ExternalInput")
    wk_d = nc.dram_tensor("wk", [P, ncb, FPC], BF16, kind="ExternalInput")
    wv_d = nc.dram_tensor("wv", [P, ncb, FPC], BF16, kind="ExternalInput")
    bqkv_d = nc.dram_tensor("bqkv", [FPC, 3], F32, kind="ExternalInput")
    bv_d = nc.dram_tensor("bv", [1, FPC], BF16, kind="ExternalInput")
    wp_d = nc.dram_tensor("wp", [P, 2, C], BF16, kind="ExternalInput")
    triu_d = nc.dram_tensor("triu", [P, P], BF16, kind="ExternalInput")
    out_d = nc.dram_tensor("out", [t_len, C], BF16, kind="ExternalOutput")

    from contextlib import ExitStack

    with tile.TileContext(nc) as tc, ExitStack() as ctx:
        consts = ctx.enter_context(tc.tile_pool(name="consts", bufs=1))
        bigs = ctx.enter_context(tc.tile_pool(name="bigs", bufs=1))
        xts = ctx.enter_context(tc.tile_pool(name="xts", bufs=1))
        qkts = ctx.enter_context(tc.tile_pool(name="qkts", bufs=1))
        yts = ctx.enter_context(tc.tile_pool(name="yts", bufs=1))
        exps = ctx.enter_context(tc.tile_pool(name="exps", bufs=1))
        smalls = ctx.enter_context(tc.tile_pool(name="smalls", bufs=2))
        stage = ctx.enter_context(tc.tile_pool(name="stage", bufs=4))
        psum = ctx.enter_context(tc.tile_pool(name="psum", bufs=2, space="PSUM"))

        # PE warm-up with NO DMA dependency (memset feeds it): opens the HAM
        # activity window during the NEFF-init barrier + first input DMAs.
        # Warm matmuls go to the st2 psum ring (idle until attention starts)
        # so interleaving them with DMA-gated QKV chains can't deadlock the
        # qkv psum ring.
        warm = consts.tile([P, P], BF16)
        nc.vector.memset(warm, 0.125)

        def emit_warm(n):
            for _ in range(n):
                wps = psum.tile([P, P], F32, tag="st2", name="warm_ps")
                nc.tensor.matmul(wps, warm, warm, start=True, stop=True)

        emit_warm(34)

        triu2 = consts.tile([P, 2, P], BF16)
        nc.sync.dma_start(out=triu2[:, 0, :], in_=triu_d.ap())
        nc.sync.dma_start(out=triu2[:, 1, :], in_=triu_d.ap())
        bq3 = consts.tile([P, 2, 3], F32)
        nc.sync.dma_start(out=bq3, in_=bqkv_d.ap().rearrange("(b p) c -> p b c", p=P))
        bv = consts.tile([1, FPC], BF16)
        nc.sync.dma_start(out=bv, in_=bv_d.ap())

        # ---- bulk input DMAs, contiguous, split across both hwdge queues ----
        wk_sb = bigs.tile([P, ncb, FPC], BF16, tag="wk")
        wq_sb = bigs.tile([P, ncb, FPC], BF16, tag="wq")
        nc.sync.dma_start(out=wk_sb, in_=wk_d.ap())
        nc.scalar.dma_start(out=wq_sb, in_=wq_d.ap())
        # x^T streamed in query-chunk column order so qc0's QKV (which only
        # reads xt[:, 0:QCW]) can start after ~1MB instead of ~4MB
        xt = [xts.tile([P, t_len], BF16, tag=f"xt{i}", name=f"xt{i}") for i in range(ncb)]
        for cb in range(ncb):
            eng = nc.sync if cb % 2 == 0 else nc.scalar
            eng.dma_start(
                out=xt[cb][:, 0:QCW],
                in_=xT_d.ap()[cb * P : (cb + 1) * P, 0:QCW],
            )
        wv_sb = bigs.tile([P, ncb, FPC], BF16, tag="wv")
        nc.sync.dma_start(out=wv_sb, in_=wv_d.ap())
        wp_sb = bigs.tile([P, 2, C], BF16, tag="wp")
        nc.scalar.dma_start(out=wp_sb, in_=wp_d.ap())
        for qcc in range(1, nqc):
            for cb in range(ncb):
                nc.sync.dma_start(
                    out=xt[cb][:, qcc * QCW : (qcc + 1) * QCW],
                    in_=xT_d.ap()[cb * P : (cb + 1) * P, qcc * QCW : (qcc + 1) * QCW],
                )
        vrep = consts.tile([P, FPC], BF16)
        nc.gpsimd.dma_start(out=vrep, in_=bv[0:1, None, :].broadcast_to([1, P, FPC]))

        qt = [qkts.tile([P, t_len], BF16, tag=f"qt{i}", name=f"qt{i}") for i in range(2)]
        kt = [qkts.tile([P, t_len], BF16, tag=f"kt{i}", name=f"kt{i}") for i in range(2)]
        # V stationaries, per (token-tile, pair): statA = [V_A(64) | ones(64)]
        # cols 0:128, statB = [ones(64) | V_B(64)] cols 128:256.  The ones
        # columns replicate each head's softmax denominator across 64 PSUM
        # partitions for free (matmul cost is independent of M).
        v_sb = bigs.tile([P, nt, 2, 2 * P], BF16, tag="v")
        nc.vector.memset(v_sb[:, :, :, 64:192], 1.0)
        yt = [yts.tile([P, t_len], BF16, tag=f"yt{i}", name=f"yt{i}") for i in range(2)]

        # ---------- emission helpers ----------
        def emit_qk_chunk(widx, wsb, dst, pair, qc, warm_every=0):
            """One [128, QCW] chunk of Q^T or K^T (8 accumulating matmuls +
            fused bias/scale evacuation).  warm_every sprinkles HAM-keepalive
            matmuls after each DMA-gated chain matmul during startup."""
            fs = slice(pair * P, (pair + 1) * P)
            cs = slice(qc * QCW, (qc + 1) * QCW)
            ps = psum.tile([P, QCW], F32, tag="st", name="qk_ps")
            for cb in range(ncb):
                nc.tensor.matmul(
                    ps,
                    wsb[:, cb, fs],
                    xt[cb][:, cs],
                    start=(cb == 0),
                    stop=(cb == ncb - 1),
                )
                if warm_every:
                    emit_warm(warm_every)
            bias_ap = bq3[:, pair, widx : widx + 1]
            if widx == 0:  # Q: (q + b) * 1/sqrt(HD)
                nc.vector.tensor_scalar(dst[pair][:, cs], ps, bias_ap, 0.125, ADD, MULT)
            else:
                nc.vector.tensor_scalar_add(dst[pair][:, cs], ps, bias_ap)

        def emit_v_tile(t):
            """V for token tile t -> the two [128,128] PV stationaries."""
            ps = psum.tile([P, FPC], F32, tag="st", name="v_ps")
            for cb in range(ncb):
                nc.tensor.matmul(
                    ps,
                    xt[cb][:, t * P : (t + 1) * P],
                    wv_sb[:, cb, :],
                    start=(cb == 0),
                    stop=(cb == ncb - 1),
                )
            psv = ps.rearrange("p (a h d) -> p a h d", a=2, d=64)
            vrv = vrep.rearrange("p (a h d) -> p a h d", a=2, d=64)
            nc.vector.tensor_add(v_sb[:, t, :, 0:64], psv[:, :, 0, :], vrv[:, :, 0, :])
            nc.vector.tensor_add(
                v_sb[:, t, :, 192:256], psv[:, :, 1, :], vrv[:, :, 1, :]
            )

        def emit_proj_t(t):
            """Partial output projection + DMA out for one token tile."""
            ost = stage.tile([P, C], BF16, tag="ost", name="ost")
            for nch in range(2):
                ps = psum.tile([P, QCW], F32, tag="st", name="proj_ps")
                for fb in range(2):
                    nc.tensor.matmul(
                        ps,
                        yt[fb][:, t * P : (t + 1) * P],
                        wp_sb[:, fb, nch * QCW : (nch + 1) * QCW],
                        start=(fb == 0),
                        stop=(fb == 1),
                    )
                if t >= nt - 5 and nch == 1:
                    # tail tiles run after the final exps: the scalar engine
                    # is idle, so split the two casts across scalar+vector
                    nc.scalar.copy(out=ost[:, nch * QCW : (nch + 1) * QCW], in_=ps)
                else:
                    nc.vector.tensor_copy(
                        out=ost[:, nch * QCW : (nch + 1) * QCW], in_=ps
                    )
            # the last couple of output tiles go out on the (by then idle)
            # scalar queue so the tail isn't serialized on one hwdge ring
            eng = nc.scalar if t >= nt - 2 else nc.sync
            eng.dma_start(out=out_d.ap()[t * P : (t + 1) * P, :], in_=ost)

        # Attention pipeline state: at most one un-flushed (S emitted, exp/PV
        # pending) k-tile unit, so S(ki+1) runs on the PE while exp(ki) runs
        # on the scalar engine.
        pending = []
        grp = {}
        chunkq = []  # (qc_tag, thunk) deferred QKV PE work
        projq = []  # ready proj-tile thunks, pumped during the last qc

        def emit_s(pair, qc, ki):
            """Score matmuls for one 128-row k-tile: both heads into one
            2-bank PSUM tile (head A cols 0:QCW, head B cols QCW:2QCW),
            concurrent via PE row-tiling (K=64 each)."""
            cs0 = qc * QCW
            m = ki - tpq * qc
            lo = max(m, 0) * P  # first unmasked query column of this k-tile
            ks = slice(ki * P, (ki + 1) * P)
            stAB = psum.tile([P, 2 * QCW], F32, tag="st2", name="stAB")
            nc.tensor.matmul(
                stAB[:, lo:QCW],
                kt[pair][0:64, ks],
                qt[pair][0:64, cs0 + lo : cs0 + QCW],
                start=True,
                stop=True,
            )
            nc.tensor.matmul(
                stAB[:, QCW : 2 * QCW - lo],
                kt[pair][64:P, ks],
                qt[pair][64:P, cs0 + lo : cs0 + QCW],
                start=True,
                stop=True,
                tile_position=(64, 0),
            )
            pending.append((pair, qc, ki, stAB, lo, m))

        def flush_one():
            """exp + mask + PV (+ normalization at group end) for the oldest
            pending k-tile."""
            pair, qc, ki, stAB, lo, m = pending.pop(0)
            nki = tpq * (qc + 1)
            cs = slice(qc * QCW, (qc + 1) * QCW)
            # static per-ki buffer: reuse distance is a whole group, so the
            # scalar engine never waits on (or syncs against) pool rotation
            eAB = exps.tile([P, 2 * QCW], BF16, tag=f"exp{ki}", name="eAB")
            # single exp over both heads; head B is packed at [QCW, 2QCW-lo)
            # so the exp range is contiguous with no junk columns.
            nc.scalar.activation(
                eAB[:, lo : 2 * QCW - lo], stAB[:, lo : 2 * QCW - lo], EXP
            )
            if m >= 0:  # diagonal 128-block: causal triangle mask, both heads
                # two engines in parallel so the exp->PV latency stays short
                nc.vector.tensor_mul(
                    eAB[:, lo : lo + P], eAB[:, lo : lo + P], triu2[:, 0, :]
                )
                nc.gpsimd.tensor_mul(
                    eAB[:, QCW : QCW + P], eAB[:, QCW : QCW + P], triu2[:, 1, :]
                )
            if ki == 0:
                grp["yA"] = psum.tile([P, QCW], F32, tag="y", name="yA")
                grp["yB"] = psum.tile([P, QCW], F32, tag="y", name="yB")
            st, sp = ki == 0, ki == nki - 1
            # yA rows 0:64 = y_headA, rows 64:128 = denominator replicated;
            # yB rows 0:64 = denominator replicated, rows 64:128 = y_headB.
            nc.tensor.matmul(
                grp["yA"][:, lo:], v_sb[:, ki, pair, 0:P], eAB[:, lo:QCW],
                start=st, stop=sp,
            )
            nc.tensor.matmul(
                grp["yB"][:, lo:], v_sb[:, ki, pair, P : 2 * P],
                eAB[:, QCW : 2 * QCW - lo],
                start=st, stop=sp,
            )
            if sp:
                # normalize straight out of PSUM.  reciprocal_approx_fast only
                # works at base partition 0, so: realign head A's replicated
                # denominator down with a plain shifted copy (proven on DVE),
                # recip at base 0, aligned muls; head B's denominator is
                # already at base 0, its reciprocal gets shifted up instead.
                rec = smalls.tile([64, 2 * QCW], F32, tag="rec", name="rec")
                rc2 = smalls.tile([P, QCW], F32, tag="rc2", name="rc2")
                nc.vector.tensor_copy(out=rec[:, QCW:], in_=grp["yA"][64:P, :])
                nc.vector.reciprocal_approx_fast(
                    out=rec[:, 0:QCW], in_=rec[:, QCW:]
                )
                nc.vector.tensor_mul(
                    yt[pair][0:64, cs], grp["yA"][0:64, :], rec[:, 0:QCW]
                )
                nc.vector.reciprocal_approx_fast(
                    out=rec[:, QCW:], in_=grp["yB"][0:64, :]
                )
                nc.vector.tensor_copy(out=rc2[64:P, :], in_=rec[:, QCW:])
                nc.vector.tensor_mul(
                    yt[pair][64:P, cs], grp["yB"][64:P, :], rc2[64:P, :]
                )
                for t in range(qc * tpq, (qc + 1) * tpq):
                    if pair == 1:  # both pairs' yt chunks now ready
                        projq.append(lambda tt=t: emit_proj_t(tt))

        # ---------- main schedule ----------
        # chunkq holds deferred next-qc QKV work; projq holds ready proj
        # tiles.  QKV is pumped as soon as possible (the attention of qc+1
        # needs it), while proj tiles are held back and pumped during the
        # LAST query chunk, whose attention is otherwise exp-rate-limited
        # with no other independent PE work (that idle was re-throttling
        # HAM to half clock for much of the second half of the kernel).
        punit = [0]

        def pump(qc):
            punit[0] += 1
            if chunkq:
                chunkq.pop(0)[1]()
            elif len(projq) > 4 and qc == nqc - 1 and punit[0] % 2 == 0:
                # proj tiles are pumped at half rate so they last the whole
                # exp-rate-limited final chunk, with a couple in reserve to
                # fill the PE while the final normalization chain runs
                projq.pop(0)()

        def drain_kqv(qc):
            """Emit any still-queued chunks tagged <= qc (attention of qc
            reads their kt/qt/v_sb output, so program order must have them
            first)."""
            rest = []
            for tag, thunk in chunkq:
                if tag <= qc:
                    thunk()
                else:
                    rest.append((tag, thunk))
            chunkq[:] = rest

        # qc0 is special-cased so attention starts as soon as pair0's K/Q
        # land: V tiles are interleaved between pair0's units (v_sb[ki] is
        # emitted before flush_one(ki) reads it), and pair1's K/Q chunks are
        # deferred until right before pair1's units.  HAM-keepalive matmuls
        # bridge the DMA-gated chains.
        emit_qk_chunk(1, wk_sb, kt, 0, 0, warm_every=3)
        emit_qk_chunk(0, wq_sb, qt, 0, 0, warm_every=1)
        emit_warm(8)
        for ki in range(tpq):
            emit_s(0, 0, ki)
            emit_v_tile(ki)
            if len(pending) > 1:
                flush_one()
        emit_qk_chunk(1, wk_sb, kt, 1, 0)
        emit_qk_chunk(0, wq_sb, qt, 1, 0)
        for pair in range(2):
            chunkq.append((1, lambda p=pair: emit_qk_chunk(1, wk_sb, kt, p, 1)))
        for pair in range(2):
            chunkq.append((1, lambda p=pair: emit_qk_chunk(0, wq_sb, qt, p, 1)))
        for t in range(tpq, 2 * tpq):
            chunkq.append((1, lambda tt=t: emit_v_tile(tt)))
        for ki in range(tpq):
            emit_s(1, 0, ki)
            pump(0)
            if len(pending) > 1:
                flush_one()
        drain_kqv(1)

        for qc in range(1, nqc):
            if qc + 1 < nqc:  # queue next chunk's QKV for interleaving
                q2 = qc + 1
                for pair in range(2):
                    chunkq.append(
                        (q2, lambda p=pair: emit_qk_chunk(1, wk_sb, kt, p, q2))
                    )
                for pair in range(2):
                    chunkq.append(
                        (q2, lambda p=pair: emit_qk_chunk(0, wq_sb, qt, p, q2))
                    )
                for t in range(q2 * tpq, (q2 + 1) * tpq):
                    chunkq.append((q2, lambda tt=t: emit_v_tile(tt)))
            for pair in range(2):
                for ki in range(tpq * (qc + 1)):
                    emit_s(pair, qc, ki)
                    pump(qc)
                    if len(pending) > 1:
                        flush_one()
            if qc + 1 < nqc:
                drain_kqv(qc + 1)
        while pending:
            flush_one()
        while chunkq:
            pump(nqc - 1)
        while projq:
            projq.pop(0)()

    nc.compile()
    return nc


_NC_CACHE: dict = {}
LAST_RESULT = None


def kernel(x, w_attn, b_attn, w_proj, b_proj):
    global LAST_RESULT
    import ml_dtypes

    bf16 = ml_dtypes.bfloat16
    x = np.asarray(x, np.float32)
    w_attn = np.asarray(w_attn, np.float32)
    b_attn = np.asarray(b_attn, np.float32)
    w_proj = np.asarray(w_proj, np.float32)
    b_proj = np.asarray(b_proj, np.float32)

    if "nc" not in _NC_CACHE:
        _NC_CACHE["nc"] = build_nc(T)
    nc = _NC_CACHE["nc"]

    triu = np.triu(np.ones((P, P), np.float32)).astype(bf16)
    x_bf = x.astype(bf16)

    def permute_w(w):  # [1024, 256] -> [128, 8, 256] (cb on the free axis)
        return np.ascontiguousarray(
            w.reshape(8, P, FPC).transpose(1, 0, 2)
        ).astype(bf16)

    in_maps = []
    for core in range(8):
        b, g = core // 4, core % 4
        f0 = g * FPC
        bqkv = np.stack(
            [
                b_attn[f0 : f0 + FPC],
                b_attn[C + f0 : C + f0 + FPC],
                b_attn[2 * C + f0 : 2 * C + f0 + FPC],
            ],
            axis=1,
        ).astype(np.float32)
        in_maps.append(
            {
                "xT": np.ascontiguousarray(x_bf[b].T),
                "wq": permute_w(w_attn[:, f0 : f0 + FPC]),
                "wk": permute_w(w_attn[:, C + f0 : C + f0 + FPC]),
                "wv": permute_w(w_attn[:, 2 * C + f0 : 2 * C + f0 + FPC]),
                "bqkv": np.ascontiguousarray(bqkv),
                "bv": np.ascontiguousarray(
                    b_attn[None, 2 * C + f0 : 2 * C + f0 + FPC]
                ).astype(bf16),
                "wp": np.ascontiguousarray(
                    w_proj[f0 : f0 + FPC, :].reshape(2, P, C).transpose(1, 0, 2)
                ).astype(bf16),
                "triu": triu,
            }
        )

    trace = bool(os.environ.get("BASS_TRACE"))
    res = run_bass_kernel_spmd(
        nc,
        in_maps,
        core_ids=list(range(8)),
        trace=trace,
        tmpdir=os.environ.get("KERNEL_TRACE_DIR") or None,
    )
    LAST_RESULT = res

    y = np.empty((B, T, C), np.float32)
    for b in range(B):
        acc = res.results[4 * b]["out"].astype(np.float32)
        for g in range(1, 4):
            acc = acc + res.results[4 * b + g]["out"].astype(np.float32)
        y[b] = acc + b_proj[None, :]
    return y
